# revision 2
# baseline (speedup 1.0000x reference)
"""ChemConv Bass kernel for 8 TRN2 NeuronCores.

Math: the reference
    node_connection[a,f,i] = sum_n conn[a,n,f] * x[n,i]
    bond_score[a,o,f]      = sum_i node_connection[a,f,i] * pf[o,f,i]
    out[a,o] = sum_f bond_score[a,o,f]*bf[o,f,0] + sum_{f,c} bp[a,f,c]*bf[o,f,1+c]
is computed in "Z-form":
    Z_f[i, a]  = sum_n x[n, i] * conn[a, n, f]     (conn is the streamed moving
                                                    operand; x blocks stationary)
    out[o, a]  = sum_f W2_f[i, o]^T @ Z_f + bond term,  W2[o,f,i] = pf*bf[...,0]

Sharding: atoms (dim a) row-slabs of 256 across 8 cores; x/filters replicated.
Each core streams its 25.2 MB conn slab once (the memory roofline).

Layout: conn is pre-packed host-side to [128, KC*AS] where column
((f*16+nb)*256 + a) holds conn[a, nb*128+p, f] for partition p.  A batch DMA
of B chunks then lands 128 descriptors of B KB contiguous per partition
(vs 1 KB in a [K, AS] layout, which is descriptor-overhead-bound at ~190 GB/s).
All matmul operands are tagged float32r so the PE runs single-pass full-rate
(plain fp32 lowers to the half-rate two-pass LOW/HIGH mode).
Z_f accumulates in PSUM over 16 n-block matmuls; after each f, Z_f is copied
to SBUF and immediately folded into the out accumulator, so the tensor work
trails the conn stream by one chunk and the tail is short.
"""

import numpy as np

import concourse.bass as bass
import concourse.tile as tile
from concourse import bacc, mybir
from concourse.bass_utils import run_bass_kernel_spmd

A = 2048
IN_DEPTH = 64
OUT_DEPTH = 64
F = 12
NCORES = 8
AS = A // NCORES          # 256 atoms per core
KP = 128                  # contraction rows per matmul chunk (partition dim)
NBLK = A // KP            # 16 n-blocks
KC = F * NBLK             # 192 chunks, kc = f*16 + nb (f-major)
K = KC * KP               # 24576 total contraction length
KB = 2 * F                # bond-term contraction length (f,c) = 24

MM_DT = mybir.dt.float32r  # fp32 bits, full-rate single-pass PE streaming
F32 = mybir.dt.float32

_cache = {}


def _build_nc(B=32, bufs=3):
    """Build the per-core kernel.

    B: K-chunks per DMA batch (128 partitions x B KB per transfer)
    bufs: conn stream-pool buffering depth
    """
    nc = bacc.Bacc("TRN2", target_bir_lowering=False, debug=False)

    conn_t = nc.dram_tensor("conn_t", [KP, KC * AS], MM_DT, kind="ExternalInput").ap()
    # bond_t [24, AS] and bf2 [24, O] packed side by side -> one DMA
    bpack = nc.dram_tensor("bpack", [KB, AS + OUT_DEPTH], F32, kind="ExternalInput").ap()
    # x blocks: xpack[p, nb*64+i] = x[nb*128+p, i]
    xpack = nc.dram_tensor("xpack", [KP, NBLK * IN_DEPTH], MM_DT, kind="ExternalInput").ap()
    # w2[i, f*64+o] = pf[o,f,i] * bf[o,f,0]
    w2 = nc.dram_tensor("w2", [IN_DEPTH, F * OUT_DEPTH], MM_DT, kind="ExternalInput").ap()
    out_t = nc.dram_tensor("out_t", [OUT_DEPTH, AS], F32, kind="ExternalOutput").ap()

    # conn DMA batch sizes: big batches for bandwidth, tapered tail so the
    # final chunks (which gate the output) aren't stuck behind a 4 MB transfer
    batches = [B] * (KC // B - 1) + [B // 2, B // 4, B // 4]
    assert sum(batches) == KC
    starts = [sum(batches[:i]) for i in range(len(batches))]
    chunk_bt = []
    for bt, bsz in enumerate(batches):
        chunk_bt += [bt] * bsz

    with tile.TileContext(nc) as tc:
        with (
            tc.tile_pool(name="const", bufs=1) as cpool,
            tc.tile_pool(name="stream", bufs=bufs) as spool,
            tc.tile_pool(name="zsb", bufs=3) as zpool,
            tc.tile_pool(name="zpsum", bufs=2, space="PSUM") as zpp,
            tc.tile_pool(name="apsum", bufs=1, space="PSUM") as apool,
        ):
            # small input DMAs on the second HWDGE ring (ACT) so the conn
            # stream owns the SP ring from t=0
            x_sb = cpool.tile([KP, NBLK * IN_DEPTH], MM_DT)
            nc.scalar.dma_start(x_sb[:], xpack[:])
            w2_sb = cpool.tile([IN_DEPTH, F * OUT_DEPTH], MM_DT)
            nc.scalar.dma_start(w2_sb[:], w2[:])
            bp_sb = cpool.tile([KB, AS + OUT_DEPTH], F32)
            nc.scalar.dma_start(bp_sb[:], bpack[:])
            bond_sb = bp_sb[:, :AS]
            bf2_sb = bp_sb[:, AS:AS + OUT_DEPTH]

            ctiles = {}

            def issue_conn(bt):
                bsz = batches[bt]
                ctile = spool.tile([KP, bsz * AS], MM_DT, tag="conn",
                                   name=f"conn_{bt}")
                nc.sync.dma_start(
                    ctile[:],
                    conn_t[:, starts[bt] * AS:(starts[bt] + bsz) * AS])
                ctiles[bt] = ctile

            pre_issue = 2
            for bt in range(pre_issue):
                issue_conn(bt)

            acc = apool.tile([OUT_DEPTH, AS], F32, tag="acc")
            # bond term opens the out PSUM accumulation group
            nc.tensor.matmul(acc[:], bf2_sb[:], bond_sb[:], start=True, stop=False)

            for f in range(F):
                zps = zpp.tile([IN_DEPTH, AS], F32, tag="zps")
                for nb in range(NBLK):
                    kc = f * NBLK + nb
                    bt = chunk_bt[kc]
                    if kc == starts[bt] and bt + pre_issue < len(batches):
                        issue_conn(bt + pre_issue)
                    j = kc - starts[bt]
                    nc.tensor.matmul(
                        zps[:],
                        x_sb[:, nb * IN_DEPTH:(nb + 1) * IN_DEPTH],
                        ctiles[bt][:, j * AS:(j + 1) * AS],
                        start=(nb == 0),
                        stop=(nb == NBLK - 1),
                    )
                z_sb = zpool.tile([IN_DEPTH, AS], MM_DT, tag="z", name=f"z_{f}")
                nc.vector.tensor_copy(z_sb[:], zps[:].bitcast(MM_DT))
                nc.tensor.matmul(
                    acc[:],
                    w2_sb[:, f * OUT_DEPTH:(f + 1) * OUT_DEPTH],
                    z_sb[:],
                    start=False,
                    stop=(f == F - 1),
                )

            out_sb = spool.tile([OUT_DEPTH, AS], F32, tag="osb")
            nc.vector.tensor_copy(out_sb[:], acc[:])
            nc.sync.dma_start(out_t[:], out_sb[:])

    nc.compile()
    return nc


def _prep(node_property_tensor, connectivity_tensor, bond_property_tensor,
          property_filters, bond_filters):
    x = np.asarray(node_property_tensor, dtype=np.float32)
    conn = np.asarray(connectivity_tensor, dtype=np.float32)
    bp = np.asarray(bond_property_tensor, dtype=np.float32)
    pf = np.asarray(property_filters, dtype=np.float32)
    bf = np.asarray(bond_filters, dtype=np.float32)

    W = pf * bf[:, :, 0:1]                                # (O, F, I)
    w2 = np.ascontiguousarray(W.transpose(2, 1, 0).reshape(IN_DEPTH, F * OUT_DEPTH))
    bf2 = np.ascontiguousarray(bf[:, :, 1:3].reshape(OUT_DEPTH, KB).T)  # (24, O)
    xpack = np.ascontiguousarray(
        x.reshape(NBLK, KP, IN_DEPTH).transpose(1, 0, 2).reshape(KP, NBLK * IN_DEPTH))

    common = {"xpack": xpack, "w2": w2}
    in_maps = []
    for c in range(NCORES):
        sl = slice(c * AS, (c + 1) * AS)
        # conn_t[p, (f*16+nb)*256 + a] = conn[a0+a, nb*128+p, f]
        cslab = conn[sl].reshape(AS, NBLK, KP, F)
        conn_c = np.ascontiguousarray(
            cslab.transpose(2, 3, 1, 0).reshape(KP, KC * AS))
        bond_tc = bp[sl].reshape(AS, KB).T              # (24, AS)
        in_maps.append({
            "conn_t": conn_c,
            "bpack": np.ascontiguousarray(
                np.concatenate([bond_tc, bf2], axis=1)),  # (24, AS + O)
            **common,
        })
    return in_maps


def kernel(node_property_tensor, connectivity_tensor, bond_property_tensor,
           property_filters, bond_filters):
    in_maps = _prep(node_property_tensor, connectivity_tensor,
                    bond_property_tensor, property_filters, bond_filters)

    if "nc" not in _cache:
        _cache["nc"] = _build_nc()
    nc = _cache["nc"]

    res = run_bass_kernel_spmd(nc, in_maps, core_ids=list(range(NCORES)))

    out = np.empty((A, OUT_DEPTH), dtype=np.float32)
    for c in range(NCORES):
        out[c * AS:(c + 1) * AS, :] = res.results[c]["out_t"].T
    return out


# revision 8
# speedup vs baseline: 1.3613x; 1.3613x over previous
"""ChemConv Bass kernel for 8 TRN2 NeuronCores.

Math: the reference
    node_connection[a,f,i] = sum_n conn[a,n,f] * x[n,i]
    bond_score[a,o,f]      = sum_i node_connection[a,f,i] * pf[o,f,i]
    out[a,o] = sum_f bond_score[a,o,f]*bf[o,f,0] + sum_{f,c} bp[a,f,c]*bf[o,f,1+c]
is computed in "Z-form":
    Z_f[i, a]  = sum_n x[n, i] * conn[a, n, f]     (conn is the streamed moving
                                                    operand; x blocks stationary)
    out[o, a]  = sum_f W2_f[i, o]^T @ Z_f + bond term,  W2[o,f,i] = pf*bf[...,0]

Sharding: atoms (dim a) row-slabs of 256 across 8 cores; x/filters replicated.
Each core streams its 25.2 MB conn slab once (the memory roofline).

Layout: conn is pre-packed host-side to [128, KC*AS] where column
((f*16+nb)*256 + a) holds conn[a, nb*128+p, f] for partition p.  A batch DMA
of B chunks then lands 128 descriptors of B KB contiguous per partition
(vs 1 KB in a [K, AS] layout, which is descriptor-overhead-bound at ~190 GB/s).
conn and x are bf16 (halves the HBM-bound stream; measured rel err ~2e-3 vs
the 2e-2 gate); the small phase-2 operands are float32r so the PE runs
single-pass full-rate (plain fp32 lowers to the two-pass LOW/HIGH mode).
Z_f accumulates in PSUM over 16 n-block matmuls; after each f, Z_f is copied
to SBUF and immediately folded into the out accumulator, so the tensor work
trails the conn stream by one chunk and the tail is short.
"""

import ml_dtypes
import numpy as np

import concourse.bass as bass
import concourse.tile as tile
from concourse import bacc, mybir
from concourse.bass_utils import run_bass_kernel_spmd

A = 2048
IN_DEPTH = 64
OUT_DEPTH = 64
F = 12
NCORES = 8
AS = A // NCORES          # 256 atoms per core
KP = 128                  # contraction rows per matmul chunk (partition dim)
NBLK = A // KP            # 16 n-blocks
KC = F * NBLK             # 192 chunks, kc = f*16 + nb (f-major)
K = KC * KP               # 24576 total contraction length
KB = 2 * F                # bond-term contraction length (f,c) = 24

MM_DT = mybir.dt.float32r  # fp32 bits, full-rate single-pass PE streaming
BF16 = mybir.dt.bfloat16   # conn stream + x: halves the HBM-bound stream;
                           # measured end-to-end rel err ~2e-3 (gate is 2e-2)
F32 = mybir.dt.float32

_cache = {}


def _build_nc(B=32, bufs=3):
    """Build the per-core kernel.

    B: K-chunks per DMA batch (128 partitions x B KB per transfer)
    bufs: conn stream-pool buffering depth
    """
    nc = bacc.Bacc("TRN2", target_bir_lowering=False, debug=False)

    conn_t = nc.dram_tensor("conn_t", [KP, KC * AS], BF16, kind="ExternalInput").ap()
    # bond_t [24, AS] and bf2 [24, O] packed side by side -> one DMA
    bpack = nc.dram_tensor("bpack", [KB, AS + OUT_DEPTH], F32, kind="ExternalInput").ap()
    # x blocks: xpack[p, nb*64+i] = x[nb*128+p, i]
    xpack = nc.dram_tensor("xpack", [KP, NBLK * IN_DEPTH], BF16, kind="ExternalInput").ap()
    # w2[i, f*64+o] = pf[o,f,i] * bf[o,f,0]
    w2 = nc.dram_tensor("w2", [IN_DEPTH, F * OUT_DEPTH], MM_DT, kind="ExternalInput").ap()
    out_t = nc.dram_tensor("out_t", [OUT_DEPTH, AS], F32, kind="ExternalOutput").ap()

    # conn DMA batch sizes: big batches for bandwidth, tapered tail so the
    # final chunks (which gate the output) aren't stuck behind a 4 MB transfer
    batches = [B] * (KC // B - 1) + [B // 2, B // 4, B // 4]
    assert sum(batches) == KC
    starts = [sum(batches[:i]) for i in range(len(batches))]
    chunk_bt = []
    for bt, bsz in enumerate(batches):
        chunk_bt += [bt] * bsz

    with tile.TileContext(nc) as tc:
        with (
            tc.tile_pool(name="const", bufs=1) as cpool,
            tc.tile_pool(name="stream", bufs=bufs) as spool,
            tc.tile_pool(name="zsb", bufs=3) as zpool,
            tc.tile_pool(name="zpsum", bufs=2, space="PSUM") as zpp,
            tc.tile_pool(name="apsum", bufs=1, space="PSUM") as apool,
        ):
            # small input DMAs on the second HWDGE ring (ACT) so the conn
            # stream owns the SP ring from t=0
            x_sb = cpool.tile([KP, NBLK * IN_DEPTH], BF16)
            nc.scalar.dma_start(x_sb[:], xpack[:])
            w2_sb = cpool.tile([IN_DEPTH, F * OUT_DEPTH], MM_DT)
            nc.scalar.dma_start(w2_sb[:], w2[:])
            bp_sb = cpool.tile([KB, AS + OUT_DEPTH], F32)
            nc.scalar.dma_start(bp_sb[:], bpack[:])
            bond_sb = bp_sb[:, :AS]
            bf2_sb = bp_sb[:, AS:AS + OUT_DEPTH]

            ctiles = {}

            def issue_conn(bt):
                bsz = batches[bt]
                ctile = spool.tile([KP, bsz * AS], BF16, tag="conn",
                                   name=f"conn_{bt}")
                nc.sync.dma_start(
                    ctile[:],
                    conn_t[:, starts[bt] * AS:(starts[bt] + bsz) * AS])
                ctiles[bt] = ctile

            pre_issue = 2
            for bt in range(pre_issue):
                issue_conn(bt)

            acc = apool.tile([OUT_DEPTH, AS], F32, tag="acc")
            # bond term opens the out PSUM accumulation group
            nc.tensor.matmul(acc[:], bf2_sb[:], bond_sb[:], start=True, stop=False)

            for f in range(F):
                zps = zpp.tile([IN_DEPTH, AS], F32, tag="zps")
                for nb in range(NBLK):
                    kc = f * NBLK + nb
                    bt = chunk_bt[kc]
                    if kc == starts[bt] and bt + pre_issue < len(batches):
                        issue_conn(bt + pre_issue)
                    j = kc - starts[bt]
                    nc.tensor.matmul(
                        zps[:],
                        x_sb[:, nb * IN_DEPTH:(nb + 1) * IN_DEPTH],
                        ctiles[bt][:, j * AS:(j + 1) * AS],
                        start=(nb == 0),
                        stop=(nb == NBLK - 1),
                    )
                z_sb = zpool.tile([IN_DEPTH, AS], MM_DT, tag="z", name=f"z_{f}")
                nc.vector.tensor_copy(z_sb[:], zps[:].bitcast(MM_DT))
                nc.tensor.matmul(
                    acc[:],
                    w2_sb[:, f * OUT_DEPTH:(f + 1) * OUT_DEPTH],
                    z_sb[:],
                    start=False,
                    stop=(f == F - 1),
                )

            out_sb = spool.tile([OUT_DEPTH, AS], F32, tag="osb")
            nc.vector.tensor_copy(out_sb[:], acc[:])
            nc.sync.dma_start(out_t[:], out_sb[:])

    nc.compile()
    return nc


def _prep(node_property_tensor, connectivity_tensor, bond_property_tensor,
          property_filters, bond_filters):
    x = np.asarray(node_property_tensor, dtype=np.float32)
    conn = np.asarray(connectivity_tensor, dtype=np.float32)
    bp = np.asarray(bond_property_tensor, dtype=np.float32)
    pf = np.asarray(property_filters, dtype=np.float32)
    bf = np.asarray(bond_filters, dtype=np.float32)

    W = pf * bf[:, :, 0:1]                                # (O, F, I)
    w2 = np.ascontiguousarray(W.transpose(2, 1, 0).reshape(IN_DEPTH, F * OUT_DEPTH))
    bf2 = np.ascontiguousarray(bf[:, :, 1:3].reshape(OUT_DEPTH, KB).T)  # (24, O)
    xpack = np.ascontiguousarray(
        x.reshape(NBLK, KP, IN_DEPTH).transpose(1, 0, 2).reshape(KP, NBLK * IN_DEPTH)
    ).astype(ml_dtypes.bfloat16)

    common = {"xpack": xpack, "w2": w2}
    in_maps = []
    for c in range(NCORES):
        sl = slice(c * AS, (c + 1) * AS)
        # conn_t[p, (f*16+nb)*256 + a] = conn[a0+a, nb*128+p, f]
        cslab = conn[sl].astype(ml_dtypes.bfloat16).reshape(AS, NBLK, KP, F)
        conn_c = np.ascontiguousarray(
            cslab.transpose(2, 3, 1, 0).reshape(KP, KC * AS))
        bond_tc = bp[sl].reshape(AS, KB).T              # (24, AS)
        in_maps.append({
            "conn_t": conn_c,
            "bpack": np.ascontiguousarray(
                np.concatenate([bond_tc, bf2], axis=1)),  # (24, AS + O)
            **common,
        })
    return in_maps


def kernel(node_property_tensor, connectivity_tensor, bond_property_tensor,
           property_filters, bond_filters):
    in_maps = _prep(node_property_tensor, connectivity_tensor,
                    bond_property_tensor, property_filters, bond_filters)

    if "nc" not in _cache:
        _cache["nc"] = _build_nc()
    nc = _cache["nc"]

    res = run_bass_kernel_spmd(nc, in_maps, core_ids=list(range(NCORES)))

    out = np.empty((A, OUT_DEPTH), dtype=np.float32)
    for c in range(NCORES):
        out[c * AS:(c + 1) * AS, :] = res.results[c]["out_t"].T
    return out


# revision 11
# speedup vs baseline: 1.5159x; 1.1135x over previous
"""ChemConv Bass kernel for 8 TRN2 NeuronCores.

Math: the reference
    node_connection[a,f,i] = sum_n conn[a,n,f] * x[n,i]
    bond_score[a,o,f]      = sum_i node_connection[a,f,i] * pf[o,f,i]
    out[a,o] = sum_f bond_score[a,o,f]*bf[o,f,0] + sum_{f,c} bp[a,f,c]*bf[o,f,1+c]
is computed in "Z-form":
    Z_f[i, a]  = sum_n x[n, i] * conn[a, n, f]     (conn is the streamed moving
                                                    operand; x blocks stationary)
    out[o, a]  = sum_f W2_f[i, o]^T @ Z_f + bond term,  W2[o,f,i] = pf*bf[...,0]

Sharding: atoms (dim a) row-slabs of 256 across 8 cores; x/filters replicated.
Each core streams its 25.2 MB conn slab once (the memory roofline).

Layout: conn is pre-packed host-side to [128, KC*AS] where column
((f*16+nb)*256 + a) holds conn[a, nb*128+p, f] for partition p.  A batch DMA
of B chunks then lands 128 descriptors of B KB contiguous per partition
(vs 1 KB in a [K, AS] layout, which is descriptor-overhead-bound at ~190 GB/s).
conn and x are bf16 (halves the HBM-bound stream; measured rel err ~2e-3 vs
the 2e-2 gate); the small phase-2 operands are float32r so the PE runs
single-pass full-rate (plain fp32 lowers to the two-pass LOW/HIGH mode).
Z_f accumulates in PSUM over 16 n-block matmuls; after each f, Z_f is copied
to SBUF and immediately folded into the out accumulator, so the tensor work
trails the conn stream by one chunk and the tail is short.
"""

import ml_dtypes
import numpy as np

import concourse.bass as bass
import concourse.tile as tile
from concourse import bacc, mybir
from concourse.bass_utils import run_bass_kernel_spmd

A = 2048
IN_DEPTH = 64
OUT_DEPTH = 64
F = 12
NCORES = 8
AS = A // NCORES          # 256 atoms per core
KP = 128                  # contraction rows per matmul chunk (partition dim)
NBLK = A // KP            # 16 n-blocks
KC = F * NBLK             # 192 chunks, kc = f*16 + nb (f-major)
K = KC * KP               # 24576 total contraction length
KB = 2 * F                # bond-term contraction length (f,c) = 24

MM_DT = mybir.dt.float32r  # fp32 bits, full-rate single-pass PE streaming
BF16 = mybir.dt.bfloat16   # conn stream + x: halves the HBM-bound stream;
                           # measured end-to-end rel err ~2e-3 (gate is 2e-2)
F32 = mybir.dt.float32

_cache = {}


def _build_nc(B=32, bufs=4):
    """Build the per-core kernel.

    B: K-chunks per DMA batch (128 partitions x B KB per transfer)
    bufs: conn stream-pool buffering depth
    """
    nc = bacc.Bacc("TRN2", target_bir_lowering=False, debug=False)

    conn_t = nc.dram_tensor("conn_t", [KP, KC * AS], BF16, kind="ExternalInput").ap()
    # bond_t [24, AS] and bf2 [24, O] packed side by side -> one DMA
    bpack = nc.dram_tensor("bpack", [KB, AS + OUT_DEPTH], F32, kind="ExternalInput").ap()
    # x blocks: xpack[p, nb*64+i] = x[nb*128+p, i]
    xpack = nc.dram_tensor("xpack", [KP, NBLK * IN_DEPTH], BF16, kind="ExternalInput").ap()
    # w2[i, f*64+o] = pf[o,f,i] * bf[o,f,0]
    w2 = nc.dram_tensor("w2", [IN_DEPTH, F * OUT_DEPTH], MM_DT, kind="ExternalInput").ap()
    out_t = nc.dram_tensor("out_t", [OUT_DEPTH, AS], F32, kind="ExternalOutput").ap()

    # conn DMA batch sizes: small head so the PE starts early, big batches in
    # the middle for bandwidth, tapered tail so the final chunks (which gate
    # the output) aren't stuck behind a large transfer
    batches = [B // 4, B - B // 4] + [B] * (KC // B - 2) + [B // 2, B // 4, B // 4]
    assert sum(batches) == KC
    starts = [sum(batches[:i]) for i in range(len(batches))]
    chunk_bt = []
    for bt, bsz in enumerate(batches):
        chunk_bt += [bt] * bsz

    with tile.TileContext(nc) as tc:
        with (
            tc.tile_pool(name="const", bufs=1) as cpool,
            tc.tile_pool(name="stream", bufs=bufs) as spool,
            tc.tile_pool(name="zsb", bufs=3) as zpool,
            tc.tile_pool(name="zpsum", bufs=2, space="PSUM") as zpp,
            tc.tile_pool(name="apsum", bufs=1, space="PSUM") as apool,
        ):
            # small input DMAs on the second HWDGE ring (ACT) so the conn
            # stream owns the SP ring from t=0
            x_sb = cpool.tile([KP, NBLK * IN_DEPTH], BF16)
            nc.scalar.dma_start(x_sb[:], xpack[:])
            w2_sb = cpool.tile([IN_DEPTH, F * OUT_DEPTH], MM_DT)
            nc.scalar.dma_start(w2_sb[:], w2[:])
            bp_sb = cpool.tile([KB, AS + OUT_DEPTH], F32)
            nc.scalar.dma_start(bp_sb[:], bpack[:])
            bond_sb = bp_sb[:, :AS]
            bf2_sb = bp_sb[:, AS:AS + OUT_DEPTH]

            ctiles = {}

            def issue_conn(bt):
                bsz = batches[bt]
                ctile = spool.tile([KP, bsz * AS], BF16, tag="conn",
                                   name=f"conn_{bt}")
                nc.sync.dma_start(
                    ctile[:],
                    conn_t[:, starts[bt] * AS:(starts[bt] + bsz) * AS])
                ctiles[bt] = ctile

            pre_issue = 2
            for bt in range(pre_issue):
                issue_conn(bt)

            acc = apool.tile([OUT_DEPTH, AS], F32, tag="acc")
            # bond term opens the out PSUM accumulation group
            nc.tensor.matmul(acc[:], bf2_sb[:], bond_sb[:], start=True, stop=False)

            # phase-2 matmuls are deferred a couple of chunks into the NEXT
            # f group: the PE queue is in-order, so emitting acc += W2_f^T@Z_f
            # right after f's chunks would stall the PE on the vector copy of
            # Z_f; deferring lets the copy overlap f+1's chunk matmuls.
            pending = None

            def flush_pending(last):
                nonlocal pending
                if pending is None:
                    return
                fp, z = pending
                nc.tensor.matmul(
                    acc[:],
                    w2_sb[:, fp * OUT_DEPTH:(fp + 1) * OUT_DEPTH],
                    z[:],
                    start=False,
                    stop=last,
                )
                pending = None

            for f in range(F):
                zps = zpp.tile([IN_DEPTH, AS], F32, tag="zps")
                for nb in range(NBLK):
                    kc = f * NBLK + nb
                    bt = chunk_bt[kc]
                    if kc == starts[bt] and bt + pre_issue < len(batches):
                        issue_conn(bt + pre_issue)
                    j = kc - starts[bt]
                    nc.tensor.matmul(
                        zps[:],
                        x_sb[:, nb * IN_DEPTH:(nb + 1) * IN_DEPTH],
                        ctiles[bt][:, j * AS:(j + 1) * AS],
                        start=(nb == 0),
                        stop=(nb == NBLK - 1),
                    )
                    if nb == 2:
                        flush_pending(False)
                z_sb = zpool.tile([IN_DEPTH, AS], MM_DT, tag="z", name=f"z_{f}")
                nc.vector.tensor_copy(z_sb[:], zps[:].bitcast(MM_DT))
                pending = (f, z_sb)
            flush_pending(True)

            out_sb = spool.tile([OUT_DEPTH, AS], F32, tag="osb")
            nc.vector.tensor_copy(out_sb[:], acc[:])
            nc.sync.dma_start(out_t[:], out_sb[:])

    nc.compile()
    return nc


def _prep(node_property_tensor, connectivity_tensor, bond_property_tensor,
          property_filters, bond_filters):
    x = np.asarray(node_property_tensor, dtype=np.float32)
    conn = np.asarray(connectivity_tensor, dtype=np.float32)
    bp = np.asarray(bond_property_tensor, dtype=np.float32)
    pf = np.asarray(property_filters, dtype=np.float32)
    bf = np.asarray(bond_filters, dtype=np.float32)

    W = pf * bf[:, :, 0:1]                                # (O, F, I)
    w2 = np.ascontiguousarray(W.transpose(2, 1, 0).reshape(IN_DEPTH, F * OUT_DEPTH))
    bf2 = np.ascontiguousarray(bf[:, :, 1:3].reshape(OUT_DEPTH, KB).T)  # (24, O)
    xpack = np.ascontiguousarray(
        x.reshape(NBLK, KP, IN_DEPTH).transpose(1, 0, 2).reshape(KP, NBLK * IN_DEPTH)
    ).astype(ml_dtypes.bfloat16)

    common = {"xpack": xpack, "w2": w2}
    in_maps = []
    for c in range(NCORES):
        sl = slice(c * AS, (c + 1) * AS)
        # conn_t[p, (f*16+nb)*256 + a] = conn[a0+a, nb*128+p, f]
        cslab = conn[sl].astype(ml_dtypes.bfloat16).reshape(AS, NBLK, KP, F)
        conn_c = np.ascontiguousarray(
            cslab.transpose(2, 3, 1, 0).reshape(KP, KC * AS))
        bond_tc = bp[sl].reshape(AS, KB).T              # (24, AS)
        in_maps.append({
            "conn_t": conn_c,
            "bpack": np.ascontiguousarray(
                np.concatenate([bond_tc, bf2], axis=1)),  # (24, AS + O)
            **common,
        })
    return in_maps


def kernel(node_property_tensor, connectivity_tensor, bond_property_tensor,
           property_filters, bond_filters):
    in_maps = _prep(node_property_tensor, connectivity_tensor,
                    bond_property_tensor, property_filters, bond_filters)

    if "nc" not in _cache:
        _cache["nc"] = _build_nc()
    nc = _cache["nc"]

    res = run_bass_kernel_spmd(nc, in_maps, core_ids=list(range(NCORES)))

    out = np.empty((A, OUT_DEPTH), dtype=np.float32)
    for c in range(NCORES):
        out[c * AS:(c + 1) * AS, :] = res.results[c]["out_t"].T
    return out
